# revision 3
# baseline (speedup 1.0000x reference)
"""LocallyConnected1d (untied-weight conv1d) on 8 Trainium2 NeuronCores.

Problem (hardcoded):
    x:      (B=128, C=64, L=1024) f32
    weight: (O=64, C=64, P=1024, K=7) f32   (untied per output position)
    bias:   (O=64, P=1024) f32
    out:    (B=128, O=64, P=1024) f32
    out[b,o,p] = sum_{c,k} xpad[b,c,p+k] * w[o,c,p,k] + bias[o,p]  (pad=3)

Sharding: sequence-parallel over P — core m owns positions [128m, 128m+128).
Each weight element is used exactly once, so this minimizes HBM traffic
(weight shard dominates); x, weight and the output are cast to bf16
(PSUM accumulation stays fp32), halving the dominant DMA traffic.

Per-core algorithm: for each input column j (134 incl 3-halo on both sides),
a K=64-contraction matmul with the x column as the stationary operand
([c=64, b=128]) and that column's untied weights as the moving operand
([c=64, (p,o) up to 448]), accumulated into PSUM banks of 8 positions
(bank = [b=128, (pr,o)=512] f32 = one 2KB bank). Bias is added by opening
each bank's accumulation group with a K=1 ones-x-bias matmul. Even/odd j
use PE row groups 0/64 (tile_position via base partitions).

Perf notes (from perfetto trace): the kernel is HBM-DMA bound (~12MB/core
at ~330GB/s); PE work (~31us warm) hides under the DMA as long as the PE
HAM clock-gate stays warm — hence the dummy warm-up matmuls during the
initial DMA wait and bank-aligned 4-tile DMA blocks that keep PE gaps
well under the ~3.4us HAM re-throttle window.
"""

import numpy as np
import ml_dtypes

BF16 = ml_dtypes.bfloat16

B = 128
C = 64
O = 64
L = 1024
KW = 7
PAD = 3
NCORES = 8
PC = L // NCORES          # positions per core = 128
NJ = PC + 2 * PAD         # input columns per core incl halo = 134
NT = NJ // 2              # column pairs (tiles) = 67
NBANK = PC // 8           # psum banks of 8 positions = 16
BANKW = 8 * O             # psum bank free width = 512 f32
NWARM = 10                # PE warm-up matmuls (HAM clock-gate)

# DMA blocks in tile-pairs: bank g needs tiles through 4g+6, so a leading
# 7-tile block then 4-tile blocks unlock exactly one more bank each.
BLOCKS = [7] + [4] * 15
assert sum(BLOCKS) == NT
BLK_T0 = np.cumsum([0] + BLOCKS).tolist()


def _blk_of_tile(t):
    for b in range(len(BLOCKS)):
        if BLK_T0[b] <= t < BLK_T0[b + 1]:
            return b
    raise AssertionError


def _nj_count(j):
    """Number of output positions input column j contributes to."""
    return min(PC - 1, j) - max(0, j - (KW - 1)) + 1


# Per-tile weight-block width (positions): even/odd j padded to a common
# width so the packed array stays rectangular; only the 6 edge tiles pad.
TILE_NP = [max(_nj_count(2 * t), _nj_count(2 * t + 1)) for t in range(NT)]
TILE_OFF = np.cumsum([0] + TILE_NP).tolist()    # position offsets per tile
TOTP = TILE_OFF[-1]                             # total packed positions
WCOLS = TOTP * O                                # weight pack columns per row


def _pack_inputs(x, weight, bias):
    """Host-side relayout into DMA/matmul-friendly per-core arrays (bf16)."""
    xp = np.zeros((B, C, L + 2 * PAD), np.float32)
    xp[:, :, PAD:PAD + L] = x
    # (C, 1030, B): column-major access per (c, j)
    xt = np.ascontiguousarray(xp.transpose(1, 2, 0)).astype(BF16)

    xpacks = []
    for m in range(NCORES):
        s = xt[:, PC * m: PC * m + NJ, :]                  # (C, NJ, B)
        s = s.reshape(C, NT, 2, B).transpose(2, 0, 1, 3)   # (h, C, NT, B)
        xpacks.append(np.ascontiguousarray(s.reshape(2 * C, NT, B)))

    # weight pack: row (h*C + c), cols = per-tile blocks of TILE_NP[t]*O,
    # entry for (t, pr, o) = w[o, c, 128m+lo+pr, j-lo-pr] if valid else 0,
    # where j = 2t+h, lo = max(0, j-6)
    j_ = np.arange(NJ)[:, None]                  # (NJ, 1)
    pr = np.arange(KW)[None, :]                  # (1, KW)
    lo = np.maximum(0, j_ - (KW - 1))            # (NJ, 1)
    hi = np.minimum(PC - 1, j_)                  # (NJ, 1)
    p_loc = lo + pr                              # (NJ, KW)
    valid = p_loc <= hi                          # (NJ, KW)
    k_ = np.clip(j_ - p_loc, 0, KW - 1)          # (NJ, KW)
    m_ = np.arange(NCORES)[:, None, None]
    p_glob = np.clip(PC * m_ + p_loc[None], 0, L - 1)      # (M, NJ, KW)
    wg = weight[:, :, p_glob, np.broadcast_to(k_, p_glob.shape)]  # (O,C,M,NJ,KW)
    wg = wg * valid[None, None, None]
    wg = np.ascontiguousarray(wg.transpose(2, 3, 1, 4, 0))  # (M, NJ, C, KW, O)
    wg = wg.astype(BF16)
    wpacks = []
    for m in range(NCORES):
        wp = np.zeros((2, C, WCOLS), BF16)
        for t in range(NT):
            c0 = TILE_OFF[t] * O
            for h in range(2):
                j = 2 * t + h
                n = _nj_count(j)
                wp[h, :, c0:c0 + n * O] = wg[m, j, :, :n, :].reshape(C, n * O)
        wpacks.append(np.ascontiguousarray(wp.reshape(2 * C, WCOLS)))

    # bias pack: [1, PC*O + B + 512]; after the bias come B ones (stationary
    # operand of the bias/zeros matmuls) and 512 zeros (moving operand of the
    # bank-clearing matmul that opens each odd-row accumulation group).
    bt = np.ascontiguousarray(bias.T)            # (L, O)
    bpacks = []
    for m in range(NCORES):
        bp = np.empty((1, PC * O + B + 8 * O), np.float32)
        bp[0, :PC * O] = bt[PC * m: PC * m + PC].reshape(-1)
        bp[0, PC * O: PC * O + B] = 1.0
        bp[0, PC * O + B:] = 0.0
        bpacks.append(bp.astype(BF16))
    return xpacks, wpacks, bpacks


_PROG = None


def _build_program():
    global _PROG
    if _PROG is not None:
        return _PROG

    import concourse.bacc as bacc
    import concourse.mybir as mybir
    import concourse.tile as tile

    F32 = mybir.dt.float32
    BF = mybir.dt.bfloat16

    nc = bacc.Bacc("TRN2", target_bir_lowering=False, debug=False,
                   num_devices=NCORES)
    x_d = nc.dram_tensor("xp", (2 * C, NT, B), BF, kind="ExternalInput")
    w_d = nc.dram_tensor("wp", (2 * C, WCOLS), BF, kind="ExternalInput")
    b_d = nc.dram_tensor("bp", (1, PC * O + B + BANKW), BF,
                         kind="ExternalInput")
    o_d = nc.dram_tensor("out", (B, PC * O), BF, kind="ExternalOutput")

    nblk = len(BLOCKS)

    with tile.TileContext(nc) as tc:
        with (
            tc.tile_pool(name="xb", bufs=4) as xpool,
            tc.tile_pool(name="wb", bufs=4) as wpool,
            tc.tile_pool(name="cst", bufs=1) as cpool,
            tc.tile_pool(name="st", bufs=4) as spool,
            tc.tile_pool(name="ps", bufs=4, space="PSUM") as ppool,
        ):
            biast = cpool.tile([1, PC * O + B + BANKW], BF)
            nc.sync.dma_start(biast[:], b_d[:])
            ones = biast[0:1, PC * O: PC * O + B]
            zeros = biast[0:1, PC * O + B: PC * O + B + BANKW]

            # PE warm-up: the HAM clock gate keeps the PE at 1.2GHz until
            # it has seen ~3.4us of sustained matmul activity. Burn the
            # initial DMA-wait (PE would idle anyway) on dummy matmuls over
            # a memset tile so the real work starts at 2.4GHz.
            warm = cpool.tile([C, 5 * B], BF)
            nc.vector.memset(warm[:], 0)
            for i in range(NWARM):
                pw = ppool.tile([B, BANKW], F32, tag="pse")
                nc.tensor.matmul(pw[:], warm[:, 0:B], warm[:, B:5 * B],
                                 start=True, stop=True)

            xtiles = {}
            wtiles = {}
            for blk in range(nblk):
                t0 = BLK_T0[blk]
                nt = BLOCKS[blk]
                xt = xpool.tile([2 * C, nt * B], BF, tag="xt")
                nc.sync.dma_start(xt[:], x_d[:, t0:t0 + nt, :])
                wc0 = TILE_OFF[t0] * O
                wc1 = TILE_OFF[t0 + nt] * O
                wt = wpool.tile([2 * C, wc1 - wc0], BF, tag="wt")
                nc.sync.dma_start(wt[:], w_d[:, wc0:wc1])
                xtiles[blk] = (t0, xt)
                wtiles[blk] = (t0, wt)

            # HW constraint (probed): matmuls with different lhsT base
            # partitions (PE row groups) must not accumulate into the same
            # PSUM bank — that crashes execution. So even-j (rows 0:64) and
            # odd-j (rows 64:128) pieces accumulate into separate banks,
            # combined by the DVE at eviction time.
            stage = None
            for g in range(NBANK):
                # last two banks get their own stage + store so the final
                # eviction->store chain after the last matmul is short
                solo = g >= NBANK - 2
                if solo:
                    stage = spool.tile([B, BANKW], BF)
                elif g % 2 == 0:
                    stage = spool.tile([B, 2 * BANKW], BF)
                pse = ppool.tile([B, BANKW], F32, tag="pse")
                pso = ppool.tile([B, BANKW], F32, tag="pso")
                # bias opens the even group, zeros open the odd group —
                # both write the full bank so every accumulating piece lands
                # on uniformly-written psum (per-instruction accumulate).
                nc.tensor.matmul(
                    pse[:],
                    ones,
                    biast[0:1, BANKW * g: BANKW * (g + 1)],
                    start=True, stop=False,
                )
                nc.tensor.matmul(
                    pso[:],
                    ones,
                    zeros,
                    start=True, stop=False,
                )
                for j in range(8 * g, 8 * g + 14):
                    t = j // 2
                    h = j % 2
                    lo = max(0, j - (KW - 1))
                    hi = min(PC - 1, j)
                    a = max(lo, 8 * g)
                    bb = min(hi, 8 * g + 7)
                    assert a <= bb
                    n = bb - a + 1
                    blk = _blk_of_tile(t)
                    t0, xt = xtiles[blk]
                    _, wt = wtiles[blk]
                    tt = t - t0
                    xs = xt[C * h: C * (h + 1), B * tt: B * (tt + 1)]
                    w0 = (TILE_OFF[t] - TILE_OFF[t0]) * O + O * (a - lo)
                    ws = wt[C * h: C * (h + 1), w0: w0 + O * n]
                    ps = pso if h else pse
                    nc.tensor.matmul(
                        ps[:, O * (a - 8 * g): O * (a - 8 * g + n)],
                        xs,
                        ws,
                        start=False,
                        stop=(j >= 8 * g + 12),
                    )
                if solo:
                    sl = stage[:]
                else:
                    sl = stage[:, BANKW * (g % 2): BANKW * (g % 2 + 1)]
                nc.vector.tensor_copy(sl, pse[:])
                nc.vector.tensor_add(sl, pso[:], sl)
                if solo:
                    nc.scalar.dma_start(
                        o_d[:, BANKW * g: BANKW * (g + 1)], stage[:])
                elif g % 2 == 1:
                    gb = g // 2
                    nc.scalar.dma_start(
                        o_d[:, 2 * BANKW * gb: 2 * BANKW * (gb + 1)], stage[:])

    nc.compile()
    _PROG = nc
    return nc


def _ensure_ntff_hook():
    """bass_utils' trace path imports antenv.axon_hooks, which this image
    lacks — if BASS_TRACE is set in the environment that import would crash.
    Install a minimal shim (ctypes into libaxon_pjrt.so; falls back to a
    no-hook stub that bass_utils handles by skipping the trace)."""
    import sys
    import types
    try:
        import antenv.axon_hooks  # noqa: F401
        return
    except ImportError:
        pass
    hook = None
    try:
        import contextlib
        import ctypes
        lib = ctypes.CDLL("/opt/axon/libaxon_pjrt.so")
        lib.axon_start_nrt_profile.argtypes = [
            ctypes.POINTER(ctypes.c_int64), ctypes.c_size_t]
        lib.axon_start_nrt_profile.restype = ctypes.c_int64
        lib.axon_stop_nrt_profile.argtypes = [ctypes.c_char_p]
        lib.axon_stop_nrt_profile.restype = ctypes.c_int64

        @contextlib.contextmanager
        def _hook(output_dir, device_ids):
            import jax
            jax.devices()
            if device_ids:
                ids = (ctypes.c_int64 * len(device_ids))(*device_ids)
                rc = lib.axon_start_nrt_profile(ids, len(device_ids))
            else:
                rc = lib.axon_start_nrt_profile(None, 0)
            if rc != 0:
                raise RuntimeError(f"axon_start_nrt_profile rc={rc}")
            try:
                yield
            finally:
                lib.axon_stop_nrt_profile(str(output_dir).encode())

        hook = _hook
    except Exception:
        hook = None
    mod = types.ModuleType("antenv.axon_hooks")
    mod.get_axon_ntff_profile_hook = lambda: hook
    mod.set_axon_ntff_profile_hook = lambda h: None
    try:
        import antenv
        antenv.axon_hooks = mod
    except ImportError:
        pass
    sys.modules["antenv.axon_hooks"] = mod


def _run(x, weight, bias, trace=False, tmpdir=None):
    from concourse.bass_utils import run_bass_kernel_spmd
    _ensure_ntff_hook()

    x = np.asarray(x, dtype=np.float32)
    weight = np.asarray(weight, dtype=np.float32)
    bias = np.asarray(bias, dtype=np.float32)
    xpacks, wpacks, bpacks = _pack_inputs(x, weight, bias)
    nc = _build_program()
    in_maps = [{"xp": xpacks[m], "wp": wpacks[m], "bp": bpacks[m]}
               for m in range(NCORES)]
    res = run_bass_kernel_spmd(nc, in_maps, list(range(NCORES)), trace=trace,
                               tmpdir=tmpdir)
    outs = [r["out"].astype(np.float32).reshape(B, PC, O).transpose(0, 2, 1)
            for r in res.results]
    full = np.ascontiguousarray(np.concatenate(outs, axis=2))
    return full, res


def kernel(x, weight, bias):
    out, _ = _run(x, weight, bias, trace=False)
    return out


# revision 4
# speedup vs baseline: 1.0895x; 1.0895x over previous
"""LocallyConnected1d (untied-weight conv1d) on 8 Trainium2 NeuronCores.

Problem (hardcoded):
    x:      (B=128, C=64, L=1024) f32
    weight: (O=64, C=64, P=1024, K=7) f32   (untied per output position)
    bias:   (O=64, P=1024) f32
    out:    (B=128, O=64, P=1024) f32
    out[b,o,p] = sum_{c,k} xpad[b,c,p+k] * w[o,c,p,k] + bias[o,p]  (pad=3)

Sharding: sequence-parallel over P — core m owns positions [128m, 128m+128).
Each weight element is used exactly once, so this minimizes HBM traffic
(weight shard dominates); x, weight and the output are cast to bf16
(PSUM accumulation stays fp32), halving the dominant DMA traffic.

Per-core algorithm (pair scheme): input columns are processed in PAIRS
(j=2t, 2t+1) stacked into the full 128-partition contraction dim
[(h, c) = 2*64]. Pair t's stationary operand is the x pair-column
[128, b=128]; for each PSUM bank g (8 output positions), the 7 pairs
t=4g..4g+6 contribute moving blocks of untied weights [128, width*64]
(width = overlap of the pair's 8-position span with the bank, summing to
32 position-instances = 2048 moving columns per bank — half the moving
columns of a 64-row scheme, which matters because the HAM clock gate
keeps the PE at 1.2GHz in this DMA-bound regime: PE work must fit under
the DMA time even when cold). Positions covered by only one column of
the pair get 64 zero rows in the weight pack (~12% pad bytes). Bias is
added by opening each bank's accumulation group with a K=1 ones-x-bias
matmul that writes the full bank.
"""

import numpy as np
import ml_dtypes

BF16 = ml_dtypes.bfloat16

B = 128
C = 64
O = 64
L = 1024
KW = 7
PAD = 3
NCORES = 8
PC = L // NCORES          # positions per core = 128
NJ = PC + 2 * PAD         # input columns per core incl halo = 134
NT = NJ // 2              # column pairs = 67
NBANK = PC // 8           # psum banks of 8 positions = 16
BANKW = 8 * O             # psum bank free width = 512 f32


def _bank_pairs(g):
    """Pairs contributing to bank g: (t, lo, hi) position ranges."""
    out = []
    for t in range(NT):
        lo = max(8 * g, 2 * t - (KW - 1), 0)
        hi = min(8 * g + 7, 2 * t + 1, PC - 1)
        if lo <= hi:
            out.append((t, lo, hi))
    return out


BANKS = [_bank_pairs(g) for g in range(NBANK)]
WOFF = {}                 # weight-pack column offset of block (g, t)
_off = 0
for _g in range(NBANK):
    for _t, _lo, _hi in BANKS[_g]:
        WOFF[(_g, _t)] = _off
        _off += (_hi - _lo + 1) * O
WCOLS = _off              # 32768
BANK_C0 = [WOFF[(g, BANKS[g][0][0])] for g in range(NBANK)] + [WCOLS]

# x DMA chunks in pairs; weight DMA chunks in banks ([2]*6 + [1]*4 keeps
# ~1MB transfers mid-kernel and a short tail).
XCHUNKS = [(0, 17), (17, 17), (34, 17), (51, 16)]
WCHUNKS = []
_g = 0
for _n in [2] * 6 + [1] * 4:
    WCHUNKS.append((_g, _n))
    _g += _n
assert _g == NBANK


def _xchunk_of(t):
    for i, (t0, n) in enumerate(XCHUNKS):
        if t0 <= t < t0 + n:
            return i, t0
    raise AssertionError


def _wchunk_of(g):
    for i, (g0, n) in enumerate(WCHUNKS):
        if g0 <= g < g0 + n:
            return i, g0
    raise AssertionError


def _pack_inputs(x, weight, bias):
    """Host-side relayout into DMA/matmul-friendly per-core arrays (bf16)."""
    xp = np.zeros((B, C, L + 2 * PAD), np.float32)
    xp[:, :, PAD:PAD + L] = x
    # (C, 1030, B): column-major access per (c, j)
    xt = np.ascontiguousarray(xp.transpose(1, 2, 0)).astype(BF16)

    xpacks = []
    for m in range(NCORES):
        s = xt[:, PC * m: PC * m + NJ, :]                  # (C, NJ, B)
        s = s.reshape(C, NT, 2, B).transpose(2, 0, 1, 3)   # (h, C, NT, B)
        xpacks.append(np.ascontiguousarray(s.reshape(2 * C, NT, B)))

    # weight pack, bank-major: block (g, t) holds, for position p in
    # [lo, hi] and row (h*C + c), w[o, c, 128m+p, 2t+h-p] (zero when
    # 2t+h-p outside [0, 7)).
    wpacks = []
    wT = np.ascontiguousarray(weight.transpose(2, 3, 1, 0))  # (L, KW, C, O)
    for m in range(NCORES):
        wp = np.zeros((2, C, WCOLS), np.float32)
        for g in range(NBANK):
            for t, lo, hi in BANKS[g]:
                off = WOFF[(g, t)]
                for h in (0, 1):
                    j = 2 * t + h
                    for p in range(lo, hi + 1):
                        k = j - p
                        if 0 <= k < KW:
                            c0 = off + (p - lo) * O
                            wp[h, :, c0:c0 + O] = wT[PC * m + p, k]
        wpacks.append(np.ascontiguousarray(
            wp.reshape(2 * C, WCOLS)).astype(BF16))

    # bias pack: [1, PC*O + B]; trailing B ones are the stationary operand
    # of the per-bank bias-opener matmuls.
    bt = np.ascontiguousarray(bias.T)            # (L, O)
    bpacks = []
    for m in range(NCORES):
        bp = np.empty((1, PC * O + B), np.float32)
        bp[0, :PC * O] = bt[PC * m: PC * m + PC].reshape(-1)
        bp[0, PC * O:] = 1.0
        bpacks.append(bp.astype(BF16))
    return xpacks, wpacks, bpacks


_PROG = None


def _build_program():
    global _PROG
    if _PROG is not None:
        return _PROG

    import concourse.bacc as bacc
    import concourse.mybir as mybir
    import concourse.tile as tile

    F32 = mybir.dt.float32
    BF = mybir.dt.bfloat16

    nc = bacc.Bacc("TRN2", target_bir_lowering=False, debug=False,
                   num_devices=NCORES)
    x_d = nc.dram_tensor("xp", (2 * C, NT, B), BF, kind="ExternalInput")
    w_d = nc.dram_tensor("wp", (2 * C, WCOLS), BF, kind="ExternalInput")
    b_d = nc.dram_tensor("bp", (1, PC * O + B), BF, kind="ExternalInput")
    o_d = nc.dram_tensor("out", (B, PC * O), BF, kind="ExternalOutput")

    with tile.TileContext(nc) as tc:
        with (
            tc.tile_pool(name="xb", bufs=4) as xpool,
            tc.tile_pool(name="wb", bufs=10) as wpool,
            tc.tile_pool(name="cst", bufs=1) as cpool,
            tc.tile_pool(name="st", bufs=4) as spool,
            tc.tile_pool(name="ps", bufs=8, space="PSUM") as ppool,
        ):
            biast = cpool.tile([1, PC * O + B], BF)
            nc.sync.dma_start(biast[:], b_d[:])
            ones = biast[0:1, PC * O: PC * O + B]

            # DMA issue order: bias, then x/w interleaved so each weight
            # chunk's pairs are already (or concurrently) in flight.
            xtiles = {}
            wtiles = {}

            def _load_x(i):
                t0, n = XCHUNKS[i]
                xt = xpool.tile([2 * C, n * B], BF, tag="xt")
                nc.sync.dma_start(xt[:], x_d[:, t0:t0 + n, :])
                xtiles[i] = xt

            def _load_w(i):
                g0, n = WCHUNKS[i]
                c0, c1 = BANK_C0[g0], BANK_C0[g0 + n]
                wt = wpool.tile([2 * C, c1 - c0], BF, tag="wt")
                nc.sync.dma_start(wt[:], w_d[:, c0:c1])
                wtiles[i] = wt

            _load_x(0)
            _load_w(0)
            _load_x(1)
            _load_w(1)
            _load_x(2)
            _load_w(2)
            _load_x(3)
            for i in range(3, len(WCHUNKS)):
                _load_w(i)

            stage = None
            for g in range(NBANK):
                # last two banks get their own stage + store so the final
                # eviction->store chain after the last matmul is short
                solo = g >= NBANK - 2
                if solo:
                    stage = spool.tile([B, BANKW], BF)
                elif g % 2 == 0:
                    stage = spool.tile([B, 2 * BANKW], BF)
                ps = ppool.tile([B, BANKW], F32, tag="ps")
                # bias opener writes the full bank (start=True) so the
                # accumulating pieces land on uniformly-written psum.
                nc.tensor.matmul(
                    ps[:],
                    ones,
                    biast[0:1, BANKW * g: BANKW * (g + 1)],
                    start=True, stop=False,
                )
                wi, g0 = _wchunk_of(g)
                wt = wtiles[wi]
                wc0 = BANK_C0[g0]
                pieces = BANKS[g]
                for idx, (t, lo, hi) in enumerate(pieces):
                    xi, t0 = _xchunk_of(t)
                    xs = xtiles[xi][:, B * (t - t0): B * (t - t0 + 1)]
                    o0 = WOFF[(g, t)] - wc0
                    ws = wt[:, o0: o0 + (hi - lo + 1) * O]
                    nc.tensor.matmul(
                        ps[:, (lo - 8 * g) * O: (hi + 1 - 8 * g) * O],
                        xs,
                        ws,
                        start=False,
                        stop=(idx == len(pieces) - 1),
                    )
                if solo:
                    sl = stage[:]
                else:
                    sl = stage[:, BANKW * (g % 2): BANKW * (g % 2 + 1)]
                nc.vector.tensor_copy(sl, ps[:])
                if solo:
                    nc.scalar.dma_start(
                        o_d[:, BANKW * g: BANKW * (g + 1)], stage[:])
                elif g % 2 == 1:
                    gb = g // 2
                    nc.scalar.dma_start(
                        o_d[:, 2 * BANKW * gb: 2 * BANKW * (gb + 1)], stage[:])

    nc.compile()
    _PROG = nc
    return nc


def _ensure_ntff_hook():
    """bass_utils' trace path imports antenv.axon_hooks, which this image
    lacks — if BASS_TRACE is set in the environment that import would crash.
    Install a minimal shim (ctypes into libaxon_pjrt.so; falls back to a
    no-hook stub that bass_utils handles by skipping the trace)."""
    import sys
    import types
    try:
        import antenv.axon_hooks  # noqa: F401
        return
    except ImportError:
        pass
    hook = None
    try:
        import contextlib
        import ctypes
        lib = ctypes.CDLL("/opt/axon/libaxon_pjrt.so")
        lib.axon_start_nrt_profile.argtypes = [
            ctypes.POINTER(ctypes.c_int64), ctypes.c_size_t]
        lib.axon_start_nrt_profile.restype = ctypes.c_int64
        lib.axon_stop_nrt_profile.argtypes = [ctypes.c_char_p]
        lib.axon_stop_nrt_profile.restype = ctypes.c_int64

        @contextlib.contextmanager
        def _hook(output_dir, device_ids):
            import jax
            jax.devices()
            if device_ids:
                ids = (ctypes.c_int64 * len(device_ids))(*device_ids)
                rc = lib.axon_start_nrt_profile(ids, len(device_ids))
            else:
                rc = lib.axon_start_nrt_profile(None, 0)
            if rc != 0:
                raise RuntimeError(f"axon_start_nrt_profile rc={rc}")
            try:
                yield
            finally:
                lib.axon_stop_nrt_profile(str(output_dir).encode())

        hook = _hook
    except Exception:
        hook = None
    mod = types.ModuleType("antenv.axon_hooks")
    mod.get_axon_ntff_profile_hook = lambda: hook
    mod.set_axon_ntff_profile_hook = lambda h: None
    try:
        import antenv
        antenv.axon_hooks = mod
    except ImportError:
        pass
    sys.modules["antenv.axon_hooks"] = mod


def _run(x, weight, bias, trace=False, tmpdir=None):
    from concourse.bass_utils import run_bass_kernel_spmd
    _ensure_ntff_hook()

    x = np.asarray(x, dtype=np.float32)
    weight = np.asarray(weight, dtype=np.float32)
    bias = np.asarray(bias, dtype=np.float32)
    xpacks, wpacks, bpacks = _pack_inputs(x, weight, bias)
    nc = _build_program()
    in_maps = [{"xp": xpacks[m], "wp": wpacks[m], "bp": bpacks[m]}
               for m in range(NCORES)]
    res = run_bass_kernel_spmd(nc, in_maps, list(range(NCORES)), trace=trace,
                               tmpdir=tmpdir)
    outs = [r["out"].astype(np.float32).reshape(B, PC, O).transpose(0, 2, 1)
            for r in res.results]
    full = np.ascontiguousarray(np.concatenate(outs, axis=2))
    return full, res


def kernel(x, weight, bias):
    out, _ = _run(x, weight, bias, trace=False)
    return out


# revision 7
# speedup vs baseline: 1.1025x; 1.0119x over previous
"""LocallyConnected1d (untied-weight conv1d) on 8 Trainium2 NeuronCores.

Problem (hardcoded):
    x:      (B=128, C=64, L=1024) f32
    weight: (O=64, C=64, P=1024, K=7) f32   (untied per output position)
    bias:   (O=64, P=1024) f32
    out:    (B=128, O=64, P=1024) f32
    out[b,o,p] = sum_{c,k} xpad[b,c,p+k] * w[o,c,p,k] + bias[o,p]  (pad=3)

Sharding: sequence-parallel over P — core m owns positions [128m, 128m+128).
Each weight element is used exactly once, so this minimizes HBM traffic
(weight shard dominates); x, weight and the output are cast to bf16
(PSUM accumulation stays fp32), halving the dominant DMA traffic.

Per-core algorithm (pair scheme): input columns are processed in PAIRS
(j=2t, 2t+1) stacked into the full 128-partition contraction dim
[(h, c) = 2*64]. Pair t's stationary operand is the x pair-column
[128, b=128]; for each PSUM bank g (8 output positions), the 7 pairs
t=4g..4g+6 contribute moving blocks of untied weights [128, width*64]
(width = overlap of the pair's 8-position span with the bank, summing to
32 position-instances = 2048 moving columns per bank — half the moving
columns of a 64-row scheme, which matters because the HAM clock gate
keeps the PE at 1.2GHz in this DMA-bound regime: PE work must fit under
the DMA time even when cold). Positions covered by only one column of
the pair get 64 zero rows in the weight pack (~12% pad bytes). Bias is
added by opening each bank's accumulation group with a K=1 ones-x-bias
matmul that writes the full bank.
"""

import numpy as np
import ml_dtypes

BF16 = ml_dtypes.bfloat16

B = 128
C = 64
O = 64
L = 1024
KW = 7
PAD = 3
NCORES = 8
PC = L // NCORES          # positions per core = 128
NJ = PC + 2 * PAD         # input columns per core incl halo = 134
NT = NJ // 2              # column pairs = 67
NBANK = PC // 8           # psum banks of 8 positions = 16
BANKW = 8 * O             # psum bank free width = 512 f32


def _bank_pairs(g):
    """Pairs contributing to bank g: (t, lo, hi) position ranges."""
    out = []
    for t in range(NT):
        lo = max(8 * g, 2 * t - (KW - 1), 0)
        hi = min(8 * g + 7, 2 * t + 1, PC - 1)
        if lo <= hi:
            out.append((t, lo, hi))
    return out


BANKS = [_bank_pairs(g) for g in range(NBANK)]
WOFF = {}                 # weight-pack column offset of block (g, t)
_off = 0
for _g in range(NBANK):
    for _t, _lo, _hi in BANKS[_g]:
        WOFF[(_g, _t)] = _off
        _off += (_hi - _lo + 1) * O
WCOLS = _off              # 32768
BANK_C0 = [WOFF[(g, BANKS[g][0][0])] for g in range(NBANK)] + [WCOLS]

# x DMA chunks in pairs; weight DMA chunks in banks (small first chunk so
# compute starts early, ~1MB mid-kernel, small last chunk for a short tail).
XCHUNKS = [(0, 17), (17, 17), (34, 17), (51, 16)]
WCHUNKS = []
_g = 0
for _n in [1] + [2] * 7 + [1]:
    WCHUNKS.append((_g, _n))
    _g += _n
assert _g == NBANK


def _xchunk_of(t):
    for i, (t0, n) in enumerate(XCHUNKS):
        if t0 <= t < t0 + n:
            return i, t0
    raise AssertionError


def _wchunk_of(g):
    for i, (g0, n) in enumerate(WCHUNKS):
        if g0 <= g < g0 + n:
            return i, g0
    raise AssertionError


def _pack_inputs(x, weight, bias):
    """Host-side relayout into DMA/matmul-friendly per-core arrays (bf16)."""
    xp = np.zeros((B, C, L + 2 * PAD), np.float32)
    xp[:, :, PAD:PAD + L] = x
    # (C, 1030, B): column-major access per (c, j)
    xt = np.ascontiguousarray(xp.transpose(1, 2, 0)).astype(BF16)

    xpacks = []
    for m in range(NCORES):
        s = xt[:, PC * m: PC * m + NJ, :]                  # (C, NJ, B)
        s = s.reshape(C, NT, 2, B).transpose(2, 0, 1, 3)   # (h, C, NT, B)
        xpacks.append(np.ascontiguousarray(s.reshape(2 * C, NT, B)))

    # weight pack, bank-major: block (g, t) holds, for position p in
    # [lo, hi] and row (h*C + c), w[o, c, 128m+p, 2t+h-p] (zero when
    # 2t+h-p outside [0, 7)).
    wpacks = []
    wT = np.ascontiguousarray(weight.transpose(2, 3, 1, 0))  # (L, KW, C, O)
    for m in range(NCORES):
        wp = np.zeros((2, C, WCOLS), np.float32)
        for g in range(NBANK):
            for t, lo, hi in BANKS[g]:
                off = WOFF[(g, t)]
                for h in (0, 1):
                    j = 2 * t + h
                    for p in range(lo, hi + 1):
                        k = j - p
                        if 0 <= k < KW:
                            c0 = off + (p - lo) * O
                            wp[h, :, c0:c0 + O] = wT[PC * m + p, k]
        wpacks.append(np.ascontiguousarray(
            wp.reshape(2 * C, WCOLS)).astype(BF16))

    # bias pack: [1, PC*O + B]; trailing B ones are the stationary operand
    # of the per-bank bias-opener matmuls.
    bt = np.ascontiguousarray(bias.T)            # (L, O)
    bpacks = []
    for m in range(NCORES):
        bp = np.empty((1, PC * O + B), np.float32)
        bp[0, :PC * O] = bt[PC * m: PC * m + PC].reshape(-1)
        bp[0, PC * O:] = 1.0
        bpacks.append(bp.astype(BF16))
    return xpacks, wpacks, bpacks


_PROG = None


def _build_program():
    global _PROG
    if _PROG is not None:
        return _PROG

    import concourse.bacc as bacc
    import concourse.mybir as mybir
    import concourse.tile as tile

    F32 = mybir.dt.float32
    BF = mybir.dt.bfloat16

    nc = bacc.Bacc("TRN2", target_bir_lowering=False, debug=False,
                   num_devices=NCORES)
    x_d = nc.dram_tensor("xp", (2 * C, NT, B), BF, kind="ExternalInput")
    w_d = nc.dram_tensor("wp", (2 * C, WCOLS), BF, kind="ExternalInput")
    b_d = nc.dram_tensor("bp", (1, PC * O + B), BF, kind="ExternalInput")
    o_d = nc.dram_tensor("out", (B, PC * O), BF, kind="ExternalOutput")

    with tile.TileContext(nc) as tc:
        with (
            tc.tile_pool(name="xb", bufs=4) as xpool,
            tc.tile_pool(name="wb", bufs=10) as wpool,
            tc.tile_pool(name="cst", bufs=1) as cpool,
            tc.tile_pool(name="st", bufs=4) as spool,
            tc.tile_pool(name="ps", bufs=8, space="PSUM") as ppool,
        ):
            biast = cpool.tile([1, PC * O + B], BF)
            nc.sync.dma_start(biast[:], b_d[:])
            ones = biast[0:1, PC * O: PC * O + B]

            # Loads alternate between the two HWDGE rings (sync + scalar):
            # consecutive DMAs on ONE ring serialize on the ~2.5us HBM
            # completion receipt at pipeline fill, and two rings let the
            # first x and w chunks land concurrently. Output stores go via
            # the gpsimd SWDGE path so they never queue behind loads.
            xtiles = {}
            wtiles = {}

            def _load_x(i, eng):
                t0, n = XCHUNKS[i]
                xt = xpool.tile([2 * C, n * B], BF, tag="xt")
                eng.dma_start(xt[:], x_d[:, t0:t0 + n, :])
                xtiles[i] = xt

            def _load_w(i, eng):
                g0, n = WCHUNKS[i]
                c0, c1 = BANK_C0[g0], BANK_C0[g0 + n]
                wt = wpool.tile([2 * C, c1 - c0], BF, tag="wt")
                eng.dma_start(wt[:], w_d[:, c0:c1])
                wtiles[i] = wt

            _load_w(0, nc.scalar)     # first bank's weights: own ring, asap
            _load_x(0, nc.sync)
            _load_w(1, nc.scalar)
            _load_x(1, nc.sync)
            _load_w(2, nc.scalar)
            _load_x(2, nc.sync)
            _load_w(3, nc.scalar)
            _load_x(3, nc.sync)
            for i in range(4, len(WCHUNKS)):
                _load_w(i, nc.sync if i % 2 == 0 else nc.scalar)

            stage = None
            for g in range(NBANK):
                # last two banks get their own stage + store so the final
                # eviction->store chain after the last matmul is short
                solo = g >= NBANK - 2
                if solo:
                    stage = spool.tile([B, BANKW], BF)
                elif g % 2 == 0:
                    stage = spool.tile([B, 2 * BANKW], BF)
                ps = ppool.tile([B, BANKW], F32, tag="ps")
                # bias opener writes the full bank (start=True) so the
                # accumulating pieces land on uniformly-written psum.
                nc.tensor.matmul(
                    ps[:],
                    ones,
                    biast[0:1, BANKW * g: BANKW * (g + 1)],
                    start=True, stop=False,
                )
                wi, g0 = _wchunk_of(g)
                wt = wtiles[wi]
                wc0 = BANK_C0[g0]
                pieces = BANKS[g]
                for idx, (t, lo, hi) in enumerate(pieces):
                    xi, t0 = _xchunk_of(t)
                    xs = xtiles[xi][:, B * (t - t0): B * (t - t0 + 1)]
                    o0 = WOFF[(g, t)] - wc0
                    ws = wt[:, o0: o0 + (hi - lo + 1) * O]
                    nc.tensor.matmul(
                        ps[:, (lo - 8 * g) * O: (hi + 1 - 8 * g) * O],
                        xs,
                        ws,
                        start=False,
                        stop=(idx == len(pieces) - 1),
                    )
                if solo:
                    sl = stage[:]
                else:
                    sl = stage[:, BANKW * (g % 2): BANKW * (g % 2 + 1)]
                nc.vector.tensor_copy(sl, ps[:])
                if solo:
                    nc.gpsimd.dma_start(
                        o_d[:, BANKW * g: BANKW * (g + 1)], stage[:])
                elif g % 2 == 1:
                    gb = g // 2
                    nc.gpsimd.dma_start(
                        o_d[:, 2 * BANKW * gb: 2 * BANKW * (gb + 1)], stage[:])

    nc.compile()
    _PROG = nc
    return nc


def _ensure_ntff_hook():
    """bass_utils' trace path imports antenv.axon_hooks, which this image
    lacks — if BASS_TRACE is set in the environment that import would crash.
    Install a minimal shim (ctypes into libaxon_pjrt.so; falls back to a
    no-hook stub that bass_utils handles by skipping the trace)."""
    import sys
    import types
    try:
        import antenv.axon_hooks  # noqa: F401
        return
    except ImportError:
        pass
    hook = None
    try:
        import contextlib
        import ctypes
        lib = ctypes.CDLL("/opt/axon/libaxon_pjrt.so")
        lib.axon_start_nrt_profile.argtypes = [
            ctypes.POINTER(ctypes.c_int64), ctypes.c_size_t]
        lib.axon_start_nrt_profile.restype = ctypes.c_int64
        lib.axon_stop_nrt_profile.argtypes = [ctypes.c_char_p]
        lib.axon_stop_nrt_profile.restype = ctypes.c_int64

        @contextlib.contextmanager
        def _hook(output_dir, device_ids):
            import jax
            jax.devices()
            if device_ids:
                ids = (ctypes.c_int64 * len(device_ids))(*device_ids)
                rc = lib.axon_start_nrt_profile(ids, len(device_ids))
            else:
                rc = lib.axon_start_nrt_profile(None, 0)
            if rc != 0:
                raise RuntimeError(f"axon_start_nrt_profile rc={rc}")
            try:
                yield
            finally:
                lib.axon_stop_nrt_profile(str(output_dir).encode())

        hook = _hook
    except Exception:
        hook = None
    mod = types.ModuleType("antenv.axon_hooks")
    mod.get_axon_ntff_profile_hook = lambda: hook
    mod.set_axon_ntff_profile_hook = lambda h: None
    try:
        import antenv
        antenv.axon_hooks = mod
    except ImportError:
        pass
    sys.modules["antenv.axon_hooks"] = mod


def _run(x, weight, bias, trace=False, tmpdir=None):
    from concourse.bass_utils import run_bass_kernel_spmd
    _ensure_ntff_hook()

    x = np.asarray(x, dtype=np.float32)
    weight = np.asarray(weight, dtype=np.float32)
    bias = np.asarray(bias, dtype=np.float32)
    xpacks, wpacks, bpacks = _pack_inputs(x, weight, bias)
    nc = _build_program()
    in_maps = [{"xp": xpacks[m], "wp": wpacks[m], "bp": bpacks[m]}
               for m in range(NCORES)]
    res = run_bass_kernel_spmd(nc, in_maps, list(range(NCORES)), trace=trace,
                               tmpdir=tmpdir)
    outs = [r["out"].astype(np.float32).reshape(B, PC, O).transpose(0, 2, 1)
            for r in res.results]
    full = np.ascontiguousarray(np.concatenate(outs, axis=2))
    return full, res


def kernel(x, weight, bias):
    out, _ = _run(x, weight, bias, trace=False)
    return out


# revision 10
# speedup vs baseline: 1.2598x; 1.1427x over previous
"""LocallyConnected1d (untied-weight conv1d) on 8 Trainium2 NeuronCores.

Problem (hardcoded):
    x:      (B=128, C=64, L=1024) f32
    weight: (O=64, C=64, P=1024, K=7) f32   (untied per output position)
    bias:   (O=64, P=1024) f32
    out:    (B=128, O=64, P=1024) f32
    out[b,o,p] = sum_{c,k} xpad[b,c,p+k] * w[o,c,p,k] + bias[o,p]  (pad=3)

Sharding: sequence-parallel over P — core m owns positions [128m, 128m+128).
Each weight element is used exactly once, so this minimizes HBM traffic
(weight shard dominates); x, weight and the output are cast to bf16
(PSUM accumulation stays fp32), halving the dominant DMA traffic.

Per-core algorithm (pair scheme): input columns are processed in PAIRS
(j=2t, 2t+1) stacked into the full 128-partition contraction dim
[(h, c) = 2*64]. Pair t's stationary operand is the x pair-column
[128, b=128]; for each PSUM bank g (8 output positions), the 7 pairs
t=4g..4g+6 contribute moving blocks of untied weights [128, width*64]
(width = overlap of the pair's 8-position span with the bank, summing to
32 position-instances = 2048 moving columns per bank — half the moving
columns of a 64-row scheme, which matters because the HAM clock gate
keeps the PE at 1.2GHz in this DMA-bound regime: PE work must fit under
the DMA time even when cold). Positions covered by only one column of
the pair get 64 zero rows in the weight pack (~12% pad bytes). Bias is
added by opening each bank's accumulation group with a K=1 ones-x-bias
matmul that writes the full bank.
"""

import numpy as np
import ml_dtypes

BF16 = ml_dtypes.bfloat16

B = 128
C = 64
O = 64
L = 1024
KW = 7
PAD = 3
NCORES = 8
PC = L // NCORES          # positions per core = 128
NJ = PC + 2 * PAD         # input columns per core incl halo = 134
NT = NJ // 2              # column pairs = 67
NBANK = PC // 8           # psum banks of 8 positions = 16
BANKW = 8 * O             # psum bank free width = 512 f32


def _bank_pairs(g):
    """Pairs contributing to bank g: (t, lo, hi) position ranges."""
    out = []
    for t in range(NT):
        lo = max(8 * g, 2 * t - (KW - 1), 0)
        hi = min(8 * g + 7, 2 * t + 1, PC - 1)
        if lo <= hi:
            out.append((t, lo, hi))
    return out


BANKS = [_bank_pairs(g) for g in range(NBANK)]
WOFF = {}                 # weight-pack column offset of block (g, t)
_off = 0
for _g in range(NBANK):
    for _t, _lo, _hi in BANKS[_g]:
        WOFF[(_g, _t)] = _off
        _off += (_hi - _lo + 1) * O
WCOLS = _off              # 32768
BANK_C0 = [WOFF[(g, BANKS[g][0][0])] for g in range(NBANK)] + [WCOLS]

# x DMA chunks in pairs; weight DMA chunks in banks (small first chunk so
# compute starts early, ~1MB mid-kernel, small last chunk for a short tail).
XCHUNKS = [(0, 17), (17, 17), (34, 17), (51, 16)]
WCHUNKS = []
_g = 0
for _n in [1] + [2] * 7 + [1]:
    WCHUNKS.append((_g, _n))
    _g += _n
assert _g == NBANK


def _xchunk_of(t):
    for i, (t0, n) in enumerate(XCHUNKS):
        if t0 <= t < t0 + n:
            return i, t0
    raise AssertionError


def _wchunk_of(g):
    for i, (g0, n) in enumerate(WCHUNKS):
        if g0 <= g < g0 + n:
            return i, g0
    raise AssertionError


def _pack_inputs(x, weight, bias):
    """Host-side relayout into DMA/matmul-friendly per-core arrays (bf16)."""
    xp = np.zeros((B, C, L + 2 * PAD), np.float32)
    xp[:, :, PAD:PAD + L] = x
    # (C, 1030, B): column-major access per (c, j)
    xt = np.ascontiguousarray(xp.transpose(1, 2, 0)).astype(BF16)

    xpacks = []
    for m in range(NCORES):
        s = xt[:, PC * m: PC * m + NJ, :]                  # (C, NJ, B)
        s = s.reshape(C, NT, 2, B).transpose(2, 0, 1, 3)   # (h, C, NT, B)
        xpacks.append(np.ascontiguousarray(s.reshape(2 * C, NT, B)))

    # weight pack, bank-major: block (g, t) holds, for position p in
    # [lo, hi] and row (h*C + c), w[o, c, 128m+p, 2t+h-p] (zero when
    # 2t+h-p outside [0, 7)).
    wpacks = []
    wT = np.ascontiguousarray(weight.transpose(2, 3, 1, 0))  # (L, KW, C, O)
    for m in range(NCORES):
        wp = np.zeros((2, C, WCOLS), np.float32)
        for g in range(NBANK):
            for t, lo, hi in BANKS[g]:
                off = WOFF[(g, t)]
                for h in (0, 1):
                    j = 2 * t + h
                    for p in range(lo, hi + 1):
                        k = j - p
                        if 0 <= k < KW:
                            c0 = off + (p - lo) * O
                            wp[h, :, c0:c0 + O] = wT[PC * m + p, k]
        wpacks.append(np.ascontiguousarray(
            wp.reshape(2 * C, WCOLS)).astype(BF16))

    # bias pack: [1, PC*O + B]; trailing B ones are the stationary operand
    # of the per-bank bias-opener matmuls.
    bt = np.ascontiguousarray(bias.T)            # (L, O)
    bpacks = []
    for m in range(NCORES):
        bp = np.empty((1, PC * O + B), np.float32)
        bp[0, :PC * O] = bt[PC * m: PC * m + PC].reshape(-1)
        bp[0, PC * O:] = 1.0
        bpacks.append(bp.astype(BF16))
    return xpacks, wpacks, bpacks


_PROG = None


def _build_program():
    global _PROG
    if _PROG is not None:
        return _PROG

    import concourse.bacc as bacc
    import concourse.mybir as mybir
    import concourse.tile as tile

    F32 = mybir.dt.float32
    BF = mybir.dt.bfloat16

    nc = bacc.Bacc("TRN2", target_bir_lowering=False, debug=False,
                   num_devices=NCORES)
    x_d = nc.dram_tensor("xp", (2 * C, NT, B), BF, kind="ExternalInput")
    w_d = nc.dram_tensor("wp", (2 * C, WCOLS), BF, kind="ExternalInput")
    b_d = nc.dram_tensor("bp", (1, PC * O + B), BF, kind="ExternalInput")
    o_d = nc.dram_tensor("out", (B, PC * O), BF, kind="ExternalOutput")

    with tile.TileContext(nc) as tc:
        with (
            tc.tile_pool(name="xb", bufs=4) as xpool,
            tc.tile_pool(name="wb", bufs=10) as wpool,
            tc.tile_pool(name="cst", bufs=1) as cpool,
            tc.tile_pool(name="st", bufs=4) as spool,
            tc.tile_pool(name="ps", bufs=8, space="PSUM") as ppool,
        ):
            biast = cpool.tile([1, PC * O + B], BF)
            nc.sync.dma_start(biast[:], b_d[:])
            ones = biast[0:1, PC * O: PC * O + B]

            # Ring assignment: ALL weight chunks stream in bank order on the
            # scalar HWDGE ring, so arrival order == the PE's consumption
            # order (mixing rings reorders arrivals and leaves the PE
            # several banks behind — a cold multi-bank tail). x + bias ride
            # the sync ring and finish early; output stores go via the
            # gpsimd SWDGE path so they never queue behind loads.
            xtiles = {}
            wtiles = {}

            def _load_x(i):
                t0, n = XCHUNKS[i]
                xt = xpool.tile([2 * C, n * B], BF, tag="xt")
                nc.sync.dma_start(xt[:], x_d[:, t0:t0 + n, :])
                xtiles[i] = xt

            def _load_w(i):
                g0, n = WCHUNKS[i]
                c0, c1 = BANK_C0[g0], BANK_C0[g0 + n]
                wt = wpool.tile([2 * C, c1 - c0], BF, tag="wt")
                nc.scalar.dma_start(wt[:], w_d[:, c0:c1])
                wtiles[i] = wt

            _load_w(0)
            _load_x(0)
            _load_w(1)
            _load_x(1)
            _load_w(2)
            _load_x(2)
            _load_x(3)
            for i in range(3, len(WCHUNKS)):
                _load_w(i)

            # Output staging: 4-bank chunks (4KB DMA rows) for the bulk,
            # then 2+1+1 so the final eviction->store chain after the last
            # matmul is short.
            STAGE_G0 = {0: 4, 4: 4, 8: 4, 12: 2, 14: 1, 15: 1}
            stage = None
            s_g0 = s_n = 0
            for g in range(NBANK):
                if g in STAGE_G0:
                    s_g0, s_n = g, STAGE_G0[g]
                    stage = spool.tile([B, s_n * BANKW], BF)
                ps = ppool.tile([B, BANKW], F32, tag="ps")
                # bias opener writes the full bank (start=True) so the
                # accumulating pieces land on uniformly-written psum.
                nc.tensor.matmul(
                    ps[:],
                    ones,
                    biast[0:1, BANKW * g: BANKW * (g + 1)],
                    start=True, stop=False,
                )
                wi, g0 = _wchunk_of(g)
                wt = wtiles[wi]
                wc0 = BANK_C0[g0]
                pieces = BANKS[g]
                for idx, (t, lo, hi) in enumerate(pieces):
                    xi, t0 = _xchunk_of(t)
                    xs = xtiles[xi][:, B * (t - t0): B * (t - t0 + 1)]
                    o0 = WOFF[(g, t)] - wc0
                    ws = wt[:, o0: o0 + (hi - lo + 1) * O]
                    nc.tensor.matmul(
                        ps[:, (lo - 8 * g) * O: (hi + 1 - 8 * g) * O],
                        xs,
                        ws,
                        start=False,
                        stop=(idx == len(pieces) - 1),
                    )
                sl = stage[:, BANKW * (g - s_g0): BANKW * (g - s_g0 + 1)]
                nc.vector.tensor_copy(sl, ps[:])
                if g == s_g0 + s_n - 1:
                    nc.gpsimd.dma_start(
                        o_d[:, BANKW * s_g0: BANKW * (s_g0 + s_n)], stage[:])

    nc.compile()
    _PROG = nc
    return nc


def _ensure_ntff_hook():
    """bass_utils' trace path imports antenv.axon_hooks, which this image
    lacks — if BASS_TRACE is set in the environment that import would crash.
    Install a minimal shim (ctypes into libaxon_pjrt.so; falls back to a
    no-hook stub that bass_utils handles by skipping the trace)."""
    import sys
    import types
    try:
        import antenv.axon_hooks  # noqa: F401
        return
    except ImportError:
        pass
    hook = None
    try:
        import contextlib
        import ctypes
        lib = ctypes.CDLL("/opt/axon/libaxon_pjrt.so")
        lib.axon_start_nrt_profile.argtypes = [
            ctypes.POINTER(ctypes.c_int64), ctypes.c_size_t]
        lib.axon_start_nrt_profile.restype = ctypes.c_int64
        lib.axon_stop_nrt_profile.argtypes = [ctypes.c_char_p]
        lib.axon_stop_nrt_profile.restype = ctypes.c_int64

        @contextlib.contextmanager
        def _hook(output_dir, device_ids):
            import jax
            jax.devices()
            if device_ids:
                ids = (ctypes.c_int64 * len(device_ids))(*device_ids)
                rc = lib.axon_start_nrt_profile(ids, len(device_ids))
            else:
                rc = lib.axon_start_nrt_profile(None, 0)
            if rc != 0:
                raise RuntimeError(f"axon_start_nrt_profile rc={rc}")
            try:
                yield
            finally:
                lib.axon_stop_nrt_profile(str(output_dir).encode())

        hook = _hook
    except Exception:
        hook = None
    mod = types.ModuleType("antenv.axon_hooks")
    mod.get_axon_ntff_profile_hook = lambda: hook
    mod.set_axon_ntff_profile_hook = lambda h: None
    try:
        import antenv
        antenv.axon_hooks = mod
    except ImportError:
        pass
    sys.modules["antenv.axon_hooks"] = mod


def _run(x, weight, bias, trace=False, tmpdir=None):
    from concourse.bass_utils import run_bass_kernel_spmd
    _ensure_ntff_hook()

    x = np.asarray(x, dtype=np.float32)
    weight = np.asarray(weight, dtype=np.float32)
    bias = np.asarray(bias, dtype=np.float32)
    xpacks, wpacks, bpacks = _pack_inputs(x, weight, bias)
    nc = _build_program()
    in_maps = [{"xp": xpacks[m], "wp": wpacks[m], "bp": bpacks[m]}
               for m in range(NCORES)]
    res = run_bass_kernel_spmd(nc, in_maps, list(range(NCORES)), trace=trace,
                               tmpdir=tmpdir)
    outs = [r["out"].astype(np.float32).reshape(B, PC, O).transpose(0, 2, 1)
            for r in res.results]
    full = np.ascontiguousarray(np.concatenate(outs, axis=2))
    return full, res


def kernel(x, weight, bias):
    out, _ = _run(x, weight, bias, trace=False)
    return out


# revision 15
# speedup vs baseline: 1.2620x; 1.0017x over previous
"""LocallyConnected1d (untied-weight conv1d) on 8 Trainium2 NeuronCores.

Problem (hardcoded):
    x:      (B=128, C=64, L=1024) f32
    weight: (O=64, C=64, P=1024, K=7) f32   (untied per output position)
    bias:   (O=64, P=1024) f32
    out:    (B=128, O=64, P=1024) f32
    out[b,o,p] = sum_{c,k} xpad[b,c,p+k] * w[o,c,p,k] + bias[o,p]  (pad=3)

Sharding: sequence-parallel over P — core m owns positions [128m, 128m+128).
Each weight element is used exactly once, so this minimizes HBM traffic
(weight shard dominates); x, weight and the output are cast to bf16
(PSUM accumulation stays fp32), halving the dominant DMA traffic.

Per-core algorithm (pair scheme): input columns are processed in PAIRS
(j=2t, 2t+1) stacked into the full 128-partition contraction dim
[(h, c) = 2*64]. Pair t's stationary operand is the x pair-column
[128, b=128]; for each PSUM bank g (8 output positions), the 7 pairs
t=4g..4g+6 contribute moving blocks of untied weights [128, width*64]
(width = overlap of the pair's 8-position span with the bank, summing to
32 position-instances = 2048 moving columns per bank — half the moving
columns of a 64-row scheme, which matters because the HAM clock gate
keeps the PE at 1.2GHz in this DMA-bound regime: PE work must fit under
the DMA time even when cold). Positions covered by only one column of
the pair get 64 zero rows in the weight pack (~12% pad bytes). Bias is
added by opening each bank's accumulation group with a K=1 ones-x-bias
matmul that writes the full bank.
"""

import numpy as np
import ml_dtypes

BF16 = ml_dtypes.bfloat16

B = 128
C = 64
O = 64
L = 1024
KW = 7
PAD = 3
NCORES = 8
PC = L // NCORES          # positions per core = 128
NJ = PC + 2 * PAD         # input columns per core incl halo = 134
NT = NJ // 2              # column pairs = 67
NBANK = PC // 8           # psum banks of 8 positions = 16
BANKW = 8 * O             # psum bank free width = 512 f32


def _bank_pairs(g):
    """Pairs contributing to bank g: (t, lo, hi) position ranges."""
    out = []
    for t in range(NT):
        lo = max(8 * g, 2 * t - (KW - 1), 0)
        hi = min(8 * g + 7, 2 * t + 1, PC - 1)
        if lo <= hi:
            out.append((t, lo, hi))
    return out


BANKS = [_bank_pairs(g) for g in range(NBANK)]
WOFF = {}                 # weight-pack column offset of block (g, t)
_off = 0
for _g in range(NBANK):
    for _t, _lo, _hi in BANKS[_g]:
        WOFF[(_g, _t)] = _off
        _off += (_hi - _lo + 1) * O
WCOLS = _off              # 32768
BANK_C0 = [WOFF[(g, BANKS[g][0][0])] for g in range(NBANK)] + [WCOLS]

# x DMA chunks in pairs; weight DMA chunks in banks (small first chunk so
# compute starts early, ~1MB mid-kernel, small last chunk for a short tail).
XCHUNKS = [(0, 17), (17, 17), (34, 17), (51, 16)]
WCHUNKS = []
_g = 0
for _n in [1] + [2] * 7 + [1]:
    WCHUNKS.append((_g, _n))
    _g += _n
assert _g == NBANK


def _xchunk_of(t):
    for i, (t0, n) in enumerate(XCHUNKS):
        if t0 <= t < t0 + n:
            return i, t0
    raise AssertionError


def _wchunk_of(g):
    for i, (g0, n) in enumerate(WCHUNKS):
        if g0 <= g < g0 + n:
            return i, g0
    raise AssertionError


def _pack_inputs(x, weight, bias):
    """Host-side relayout into DMA/matmul-friendly per-core arrays (bf16)."""
    xp = np.zeros((B, C, L + 2 * PAD), np.float32)
    xp[:, :, PAD:PAD + L] = x
    # (C, 1030, B): column-major access per (c, j)
    xt = np.ascontiguousarray(xp.transpose(1, 2, 0)).astype(BF16)

    xpacks = []
    for m in range(NCORES):
        s = xt[:, PC * m: PC * m + NJ, :]                  # (C, NJ, B)
        s = s.reshape(C, NT, 2, B).transpose(2, 0, 1, 3)   # (h, C, NT, B)
        xpacks.append(np.ascontiguousarray(s.reshape(2 * C, NT, B)))

    # weight pack, bank-major: block (g, t) holds, for position p in
    # [lo, hi] and row (h*C + c), w[o, c, 128m+p, 2t+h-p] (zero when
    # 2t+h-p outside [0, 7)).
    wpacks = []
    wT = np.ascontiguousarray(weight.transpose(2, 3, 1, 0))  # (L, KW, C, O)
    for m in range(NCORES):
        wp = np.zeros((2, C, WCOLS), np.float32)
        for g in range(NBANK):
            for t, lo, hi in BANKS[g]:
                off = WOFF[(g, t)]
                for h in (0, 1):
                    j = 2 * t + h
                    for p in range(lo, hi + 1):
                        k = j - p
                        if 0 <= k < KW:
                            c0 = off + (p - lo) * O
                            wp[h, :, c0:c0 + O] = wT[PC * m + p, k]
        wpacks.append(np.ascontiguousarray(
            wp.reshape(2 * C, WCOLS)).astype(BF16))

    # bias pack: [1, PC*O + B]; trailing B ones are the stationary operand
    # of the per-bank bias-opener matmuls.
    bt = np.ascontiguousarray(bias.T)            # (L, O)
    bpacks = []
    for m in range(NCORES):
        bp = np.empty((1, PC * O + B), np.float32)
        bp[0, :PC * O] = bt[PC * m: PC * m + PC].reshape(-1)
        bp[0, PC * O:] = 1.0
        bpacks.append(bp.astype(BF16))
    return xpacks, wpacks, bpacks


_PROG = None


def _build_program():
    global _PROG
    if _PROG is not None:
        return _PROG

    import concourse.bacc as bacc
    import concourse.mybir as mybir
    import concourse.tile as tile

    F32 = mybir.dt.float32
    BF = mybir.dt.bfloat16

    nc = bacc.Bacc("TRN2", target_bir_lowering=False, debug=False,
                   num_devices=NCORES)
    x_d = nc.dram_tensor("xp", (2 * C, NT, B), BF, kind="ExternalInput")
    w_d = nc.dram_tensor("wp", (2 * C, WCOLS), BF, kind="ExternalInput")
    b_d = nc.dram_tensor("bp", (1, PC * O + B), BF, kind="ExternalInput")
    o_d = nc.dram_tensor("out", (B, PC * O), BF, kind="ExternalOutput")

    with tile.TileContext(nc) as tc:
        with (
            tc.tile_pool(name="xb", bufs=4) as xpool,
            tc.tile_pool(name="wb", bufs=10) as wpool,
            tc.tile_pool(name="cst", bufs=1) as cpool,
            tc.tile_pool(name="st", bufs=6) as spool,
            tc.tile_pool(name="ps", bufs=8, space="PSUM") as ppool,
        ):
            biast = cpool.tile([1, PC * O + B], BF)
            nc.sync.dma_start(biast[:], b_d[:])
            ones = biast[0:1, PC * O: PC * O + B]

            # Ring assignment: ALL weight chunks stream in bank order on the
            # scalar HWDGE ring, so arrival order == the PE's consumption
            # order (mixing rings reorders arrivals and leaves the PE
            # several banks behind — a cold multi-bank tail). x + bias ride
            # the sync ring and finish early; output stores go via the
            # gpsimd SWDGE path so they never queue behind loads (mixing
            # store direction onto a load HWDGE ring crashed the device).
            xtiles = {}
            wtiles = {}

            def _load_x(i):
                t0, n = XCHUNKS[i]
                xt = xpool.tile([2 * C, n * B], BF, tag="xt")
                nc.sync.dma_start(xt[:], x_d[:, t0:t0 + n, :])
                xtiles[i] = xt

            def _load_w(i):
                g0, n = WCHUNKS[i]
                c0, c1 = BANK_C0[g0], BANK_C0[g0 + n]
                wt = wpool.tile([2 * C, c1 - c0], BF, tag="wt")
                nc.scalar.dma_start(wt[:], w_d[:, c0:c1])
                wtiles[i] = wt

            _load_w(0)
            _load_x(0)
            _load_w(1)
            _load_x(1)
            _load_w(2)
            _load_x(2)
            _load_x(3)
            for i in range(3, len(WCHUNKS)):
                _load_w(i)

            # Output staging: 4-bank chunks (4KB DMA rows) for the bulk,
            # then 2+1+1 so the final eviction->store chain after the last
            # matmul is short.
            STAGE_G0 = {0: 4, 4: 4, 8: 4, 12: 2, 14: 1, 15: 1}
            stage = None
            s_g0 = s_n = 0
            for g in range(NBANK):
                if g in STAGE_G0:
                    s_g0, s_n = g, STAGE_G0[g]
                    stage = spool.tile([B, s_n * BANKW], BF)
                ps = ppool.tile([B, BANKW], F32, tag="ps")
                # bias opener writes the full bank (start=True) so the
                # accumulating pieces land on uniformly-written psum.
                nc.tensor.matmul(
                    ps[:],
                    ones,
                    biast[0:1, BANKW * g: BANKW * (g + 1)],
                    start=True, stop=False,
                )
                wi, g0 = _wchunk_of(g)
                wt = wtiles[wi]
                wc0 = BANK_C0[g0]
                pieces = BANKS[g]
                for idx, (t, lo, hi) in enumerate(pieces):
                    xi, t0 = _xchunk_of(t)
                    xs = xtiles[xi][:, B * (t - t0): B * (t - t0 + 1)]
                    o0 = WOFF[(g, t)] - wc0
                    ws = wt[:, o0: o0 + (hi - lo + 1) * O]
                    nc.tensor.matmul(
                        ps[:, (lo - 8 * g) * O: (hi + 1 - 8 * g) * O],
                        xs,
                        ws,
                        start=False,
                        stop=(idx == len(pieces) - 1),
                    )
                sl = stage[:, BANKW * (g - s_g0): BANKW * (g - s_g0 + 1)]
                nc.vector.tensor_copy(sl, ps[:])
                if g == s_g0 + s_n - 1:
                    nc.gpsimd.dma_start(
                        o_d[:, BANKW * s_g0: BANKW * (s_g0 + s_n)], stage[:])

    nc.compile()
    _PROG = nc
    return nc


def _ensure_ntff_hook():
    """bass_utils' trace path imports antenv.axon_hooks, which this image
    lacks — if BASS_TRACE is set in the environment that import would crash.
    Install a minimal shim (ctypes into libaxon_pjrt.so; falls back to a
    no-hook stub that bass_utils handles by skipping the trace)."""
    import sys
    import types
    try:
        import antenv.axon_hooks  # noqa: F401
        return
    except ImportError:
        pass
    hook = None
    try:
        import contextlib
        import ctypes
        lib = ctypes.CDLL("/opt/axon/libaxon_pjrt.so")
        lib.axon_start_nrt_profile.argtypes = [
            ctypes.POINTER(ctypes.c_int64), ctypes.c_size_t]
        lib.axon_start_nrt_profile.restype = ctypes.c_int64
        lib.axon_stop_nrt_profile.argtypes = [ctypes.c_char_p]
        lib.axon_stop_nrt_profile.restype = ctypes.c_int64

        @contextlib.contextmanager
        def _hook(output_dir, device_ids):
            import jax
            jax.devices()
            if device_ids:
                ids = (ctypes.c_int64 * len(device_ids))(*device_ids)
                rc = lib.axon_start_nrt_profile(ids, len(device_ids))
            else:
                rc = lib.axon_start_nrt_profile(None, 0)
            if rc != 0:
                raise RuntimeError(f"axon_start_nrt_profile rc={rc}")
            try:
                yield
            finally:
                lib.axon_stop_nrt_profile(str(output_dir).encode())

        hook = _hook
    except Exception:
        hook = None
    mod = types.ModuleType("antenv.axon_hooks")
    mod.get_axon_ntff_profile_hook = lambda: hook
    mod.set_axon_ntff_profile_hook = lambda h: None
    try:
        import antenv
        antenv.axon_hooks = mod
    except ImportError:
        pass
    sys.modules["antenv.axon_hooks"] = mod


def _run(x, weight, bias, trace=False, tmpdir=None):
    from concourse.bass_utils import run_bass_kernel_spmd
    _ensure_ntff_hook()

    x = np.asarray(x, dtype=np.float32)
    weight = np.asarray(weight, dtype=np.float32)
    bias = np.asarray(bias, dtype=np.float32)
    xpacks, wpacks, bpacks = _pack_inputs(x, weight, bias)
    nc = _build_program()
    in_maps = [{"xp": xpacks[m], "wp": wpacks[m], "bp": bpacks[m]}
               for m in range(NCORES)]
    res = run_bass_kernel_spmd(nc, in_maps, list(range(NCORES)), trace=trace,
                               tmpdir=tmpdir)
    outs = [r["out"].astype(np.float32).reshape(B, PC, O).transpose(0, 2, 1)
            for r in res.results]
    full = np.ascontiguousarray(np.concatenate(outs, axis=2))
    return full, res


def kernel(x, weight, bias):
    out, _ = _run(x, weight, bias, trace=False)
    return out
